# revision 1
# baseline (speedup 1.0000x reference)
"""Randomized Hadamard transform kernel for Trainium2 (8 NeuronCores, SPMD).

Math: out = FWHT(x * seed) / sqrt(4096). The reference butterfly equals the
Sylvester Hadamard matrix, factored three ways for DMA-friendly layouts:

    H_4096 = H_16 (x) H_2 (x) H_128,   c = hi*256 + b*128 + lo

Layout trick: matmul computes out[m, n] = sum_k lhsT[k, m] * rhs[k, n].
With the *data* as stationary lhsT and the Hadamard factor as moving rhs,
one MM both contracts the data's partition index and rotates a 128-wide
free window onto the output partitions — no explicit transposes.

Per 128-row tile (rows r = r0 + rh*8 + rl, cols c = hi*256 + b*128 + lo):
  load   Lt[(rl,hi), (rh,b,lo)] <- x    (1KB-contiguous chunks)
  mult   Xt = Lt * S_rep  in fp16       (DVE/GpSimd; S_rep replicated seed)
  pass1  win=(rh,b): psum[lo, (rl,j)] = sum_(rl,hi) Xt[.., lo window] * (I8 (x) H16)
  pass2  rh: psum[(rl,j), (b',l)] = W0 @ [K2|K2] + W1 @ [K2|-K2]   (K2 = H128/64)
  store  O[(rl,j), (rh,b',l)] -> y      (512B fp16 chunks; host upcasts)
The H2 butterfly rides PSUM accumulation with a doubled-width moving rhs,
so pass2 is 2 LDW + 2 MM(N=256) per psum tile. All matmuls are fp16 (FWL).
"""

import os

import numpy as np

import concourse.mybir as mybir
from concourse import bacc
import concourse.tile as tile
from concourse.bass_utils import run_bass_kernel_spmd

N_CORES = 8
R_FULL = 8192
C = 4096
R_CORE = R_FULL // N_CORES  # 1024 rows per core
P = 128
NHI, NB, NLO, NRL, NRH = 16, 2, 128, 8, 16  # c = hi*256+b*128+lo ; r = rh*8+rl

OUT32 = os.environ.get("HAD_OUT32", "0") == "1"  # store fp32 instead of fp16


def _sylvester(n: int) -> np.ndarray:
    h = np.array([[1.0]], dtype=np.float64)
    while h.shape[0] < n:
        h = np.block([[h, h], [h, -h]])
    return h


def _consts():
    k1 = np.kron(np.eye(NRL), _sylvester(NHI)).astype(np.float16)
    k2 = _sylvester(NLO) / 64.0
    k2a = np.concatenate([k2, k2], axis=1).astype(np.float16)
    k2b = np.concatenate([k2, -k2], axis=1).astype(np.float16)
    return k1, k2a, k2b


def build_nc(rows: int = R_CORE):
    assert rows % P == 0
    n_tiles = rows // P

    k1_np, k2a_np, k2b_np = _consts()

    nc = bacc.Bacc("TRN2", target_bir_lowering=False, debug=False)
    f32 = mybir.dt.float32
    f16 = mybir.dt.float16
    out_dt = f32 if OUT32 else f16

    x_in = nc.dram_tensor("x", [rows, C], f32, kind="ExternalInput")
    s_in = nc.dram_tensor("srep", [P, C], f16, kind="ExternalInput")
    y_out = nc.dram_tensor("y", [rows, C], out_dt, kind="ExternalOutput")
    k1_dram = nc.inline_tensor(k1_np, "k1")
    k2a_dram = nc.inline_tensor(k2a_np, "k2a")
    k2b_dram = nc.inline_tensor(k2b_np, "k2b")

    with tile.TileContext(nc) as tc:
        with (
            tc.tile_pool(name="consts", bufs=1) as cpool,
            tc.tile_pool(name="lt", bufs=3) as lt_pool,
            tc.tile_pool(name="xt", bufs=3) as xt_pool,
            tc.tile_pool(name="w", bufs=2) as w_pool,
            tc.tile_pool(name="o", bufs=3) as o_pool,
            tc.tile_pool(name="ps1", bufs=4, space="PSUM") as ps1_pool,
            tc.tile_pool(name="ps2", bufs=4, space="PSUM") as ps2_pool,
        ):
            k1 = cpool.tile([P, P], f16)
            k2a = cpool.tile([P, 2 * P], f16)
            k2b = cpool.tile([P, 2 * P], f16)
            srep = cpool.tile([P, C], f16)
            # constants ride the Scalar HWDGE ring so the first x load
            # starts immediately on the Sync ring
            nc.scalar.dma_start(out=k1[:], in_=k1_dram[:])
            nc.scalar.dma_start(out=k2a[:], in_=k2a_dram[:])
            nc.scalar.dma_start(out=k2b[:], in_=k2b_dram[:])
            nc.scalar.dma_start(out=srep[:], in_=s_in[:])

            # ---- HAM warm-up: ~6us of dummy matmuls in the load-ramp
            # shadow so the PE clock-gate is at 8/8 when tile 0 arrives
            ps_warm = ps1_pool.tile([P, 512], f32, tag="ps1t")
            warm_sink = cpool.tile([P, 512], f32)
            for i in range(28):
                nc.tensor.matmul(
                    ps_warm[:, (i % 4) * P : (i % 4 + 1) * P],
                    lhsT=k1[:],
                    rhs=k1[:],
                    start=True,
                    stop=True,
                )
            nc.scalar.copy(out=warm_sink[:], in_=ps_warm[:])

            for t in range(n_tiles):
                r0 = t * P
                # ---- load whole tile: [(rl,hi), (rh, b*lo)] ; 1KB chunks
                lt = lt_pool.tile([P, C], f32)
                src = x_in[r0 : r0 + P, :].rearrange(
                    "(rh rl) (hi bl) -> rl hi rh bl", rl=NRL, bl=NB * NLO
                )
                nc.sync.dma_start(out=lt[:], in_=src)

                # ---- seed multiply + fp16 cast, chunked so pass1 windows
                # unblock progressively; 6 chunks gpsimd : 2 vector
                xq = xt_pool.tile([P, C], f16)
                for mc in range(8):
                    sl = slice(mc * 512, (mc + 1) * 512)
                    eng = nc.vector if mc % 3 == 2 else nc.gpsimd
                    eng.tensor_mul(out=xq[:, sl], in0=lt[:, sl], in1=srep[:, sl])

                # ---- pass 1: contract (rl,hi) with I8 (x) H16; lo -> partitions
                w = w_pool.tile([P, C], f16)
                for g in range(8):
                    ps = ps1_pool.tile([P, 512], f32, tag="ps1t")
                    for q in range(4):
                        win = 4 * g + q
                        nc.tensor.matmul(
                            ps[:, q * P : (q + 1) * P],
                            lhsT=xq[:, win * P : (win + 1) * P],
                            rhs=k1[:],
                            start=True,
                            stop=True,
                        )
                    wdst = w[:, g * 512 : (g + 1) * 512]
                    if g % 2 == 0:
                        nc.scalar.copy(out=wdst, in_=ps[:])
                    else:
                        nc.vector.tensor_copy(out=wdst, in_=ps[:])

                # ---- pass 2: contract lo with H128/64 + H2 butterfly via
                # accumulation; (rl,j) -> partitions
                oh = o_pool.tile([P, C], out_dt)
                for u in range(8):
                    ps = ps2_pool.tile([P, 512], f32)
                    for v in range(2):
                        rh = 2 * u + v
                        dst = ps[:, v * 256 : (v + 1) * 256]
                        nc.tensor.matmul(
                            dst,
                            lhsT=w[:, (2 * rh) * P : (2 * rh + 1) * P],
                            rhs=k2a[:],
                            start=True,
                            stop=False,
                        )
                        nc.tensor.matmul(
                            dst,
                            lhsT=w[:, (2 * rh + 1) * P : (2 * rh + 2) * P],
                            rhs=k2b[:],
                            start=False,
                            stop=True,
                        )
                    odst = oh[:, u * 512 : (u + 1) * 512]
                    if u % 2 == 1:
                        nc.vector.tensor_copy(out=odst, in_=ps[:])
                    else:
                        nc.scalar.copy(out=odst, in_=ps[:])

                # ---- store tile: 512B fp16 chunks
                dst = y_out[r0 : r0 + P, :].rearrange(
                    "(rh rl) (j bl) -> rl j rh bl", rl=NRL, bl=NB * NLO
                )
                nc.scalar.dma_start(out=dst, in_=oh[:])

    nc.compile()
    nc.finalize()
    return nc


_NC_CACHE: dict[tuple, object] = {}


def _get_nc(rows: int):
    key = (rows, OUT32)
    if key not in _NC_CACHE:
        _NC_CACHE[key] = build_nc(rows)
    return _NC_CACHE[key]


def _make_srep(seed: np.ndarray) -> np.ndarray:
    # srep[p=(rl,hi), f=(rh,bl)] = seed[hi*256+bl]; rl- and rh-independent
    return np.ascontiguousarray(
        np.tile(seed.reshape(NHI, NB * NLO), (NRL, NRH)).astype(np.float16)
    )


def run(x: np.ndarray, seed: np.ndarray, trace: bool = False):
    x = np.ascontiguousarray(x, dtype=np.float32)
    seed = np.ascontiguousarray(seed, dtype=np.float32)
    nc = _get_nc(R_CORE)
    srep = _make_srep(seed)
    in_maps = [
        {"x": x[i * R_CORE : (i + 1) * R_CORE], "srep": srep} for i in range(N_CORES)
    ]
    res = run_bass_kernel_spmd(nc, in_maps, core_ids=list(range(N_CORES)), trace=trace)
    out = np.concatenate([res.results[i]["y"] for i in range(N_CORES)], axis=0)
    if out.dtype != np.float32:
        out = out.astype(np.float32)
    return out, res


def kernel(x: np.ndarray, seed: np.ndarray) -> np.ndarray:
    out, _ = run(x, seed)
    return out

